# revision 39
# baseline (speedup 1.0000x reference)
"""Trainium2 Bass kernel for nn_BaselineRvNNModel (collapsed RvNN/TreeLSTM).

Math (reference collapses to a per-node MLP + mean pool + classifier;
edge_index is dead):
    h1 = relu(x @ W1.T + b1)                      [N, H]
    g  = h1 @ W2.T + b2                           [N, H]   (pre-LN)
    gn = (g - mu) * rsqrt(var + eps)              per-row LN core
    iou = gn @ (W_iou*ln_w).T + (W_iou@ln_b + b_wiou + b_uiou)
    i, o, u = split(iou); c = sig(i)*tanh(u); hn = sig(o)*tanh(c)
    pooled = mean_rows(hn);  out = relu(pooled @ Wc1.T + bc1) @ Wc2.T + bc2

Distribution: data-parallel over nodes, 12500 rows/core on 8 cores. Each
core emits its partial pooled sum [128, 2] f32; the host sums the 8
partials and applies the tiny classifier (256->128->4) in fp32.

Precision: mm1 (x@W1.T) and mm3 (gn@Wio.T) run as fp8-e4m3 DoubleRow
matmuls; mm2 and the LN-stats matmuls stay bf16 (mm2 in fp8 dominates
the error budget). Measured rel err ~5e-3 (tolerance 2e-2).

Scales (powers of 2): x*16, W1*16 -> pm1 = 256*xW1; h1 stored as
256*relu bf16; g = pm2/256 + b2 true-scale bf16; s8 = 8*rsqrt(var+eps);
gns = fp8(g*s8 - 8*s*mu) = 8*gn; W3*16 -> pm3 = 128*iou; gates use ACT
scale=1/128 with per-chunk bias c3.
"""

import numpy as np
import ml_dtypes

N_TOTAL = 100000
D = 768
H = 256
C = 4
NCORES = 8
LN_EPS = 1e-5

_CACHE = {}


def build_nc(npc, nt, ncores, ngroups=2):
    from contextlib import ExitStack
    import concourse.bass as bass
    import concourse.bacc as bacc
    import concourse.tile as tile
    from concourse import mybir

    f32 = mybir.dt.float32
    bf16 = mybir.dt.bfloat16
    f8 = mybir.dt.float8e4
    AF = mybir.ActivationFunctionType
    ALU = mybir.AluOpType
    DR = mybir.MatmulPerfMode.DoubleRow

    ntiles = npc // nt
    assert ntiles * nt == npc
    KD = D // 128            # 6 contraction chunks for x
    KH = H // 128            # 2 chunks for H
    NPAIR = KD // 2          # 3 DoubleRow pairs for mm1

    nc = bacc.Bacc("TRN2", target_bir_lowering=False, debug=False,
                   num_devices=ncores)

    xtt = nc.dram_tensor("xtt", [ntiles, 128, KD, nt], f8, kind="ExternalInput")
    w1d = nc.dram_tensor("w1d", [128, KD, H], f8, kind="ExternalInput")
    b1d = nc.dram_tensor("b1d", [128, KH], f32, kind="ExternalInput")   # b1*256
    w2d = nc.dram_tensor("w2d", [128, KH, H], bf16, kind="ExternalInput")
    b2d = nc.dram_tensor("b2d", [128, KH], f32, kind="ExternalInput")
    w3d = nc.dram_tensor("w3d", [128, KH, 3 * H], f8, kind="ExternalInput")
    c3d = nc.dram_tensor("c3d", [128, 3 * KH], f32, kind="ExternalInput")
    out_d = nc.dram_tensor("out", [128, KH], f32, kind="ExternalOutput")

    with tile.TileContext(nc) as tc, ExitStack() as ctx:
        # ---------------- constants (live whole kernel) ----------------
        pconst = ctx.enter_context(tc.tile_pool(name="consts", bufs=1))
        w1_sb = pconst.tile([128, KD, H], f8)
        nc.sync.dma_start(w1_sb[:], w1d.ap())
        b1_sb = pconst.tile([128, KH], f32)
        nc.sync.dma_start(b1_sb[:], b1d.ap())
        w2_sb = pconst.tile([128, KH, H], bf16)
        nc.sync.dma_start(w2_sb[:], w2d.ap())
        b2_sb = pconst.tile([128, KH], f32)
        nc.sync.dma_start(b2_sb[:], b2d.ap())
        w3_sb = pconst.tile([128, KH, 3 * H], f8)
        nc.gpsimd.dma_start(w3_sb[:], w3d.ap())
        c3_sb = pconst.tile([128, 3 * KH], f32)
        nc.gpsimd.dma_start(c3_sb[:], c3d.ap())
        ones_sb = pconst.tile([128, 32], bf16)
        nc.vector.memset(ones_sb[:], 1.0 / H)
        eps_sb = pconst.tile([16, 1], f32)
        nc.vector.memset(eps_sb[:], LN_EPS)
        ln8_sb = pconst.tile([16, 1], f32)
        nc.vector.memset(ln8_sb[:], float(np.log(8.0)))

        # persistent buffers
        pg = ctx.enter_context(tc.tile_pool(name="gbuf", bufs=1))
        gbuf = pg.tile([128, KH, npc], bf16)
        accslab = pg.tile([128, KH, 32], f32)
        nc.vector.memset(accslab[:], 0.0)

        pdram = ctx.enter_context(tc.tile_pool(name="dram", bufs=1, space="DRAM"))
        statsd = pdram.tile([ntiles, 2, nt], bf16)   # per-tile {mu, msq}
        ssd = pdram.tile([ntiles, 2, nt], bf16)      # per-tile {8s, 8*s*mu}

        # fine-grained software pipeline: stats every SG tiles, B lags A by SG
        SG = ngroups if ngroups > 2 else 5
        LAG = SG
        groups = [list(range(g, min(g + SG, ntiles)))
                  for g in range(0, ntiles, SG)]

        with tc.tile_pool(name="xin", bufs=3) as px, \
             tc.tile_pool(name="h1", bufs=3) as ph1, \
             tc.tile_pool(name="gsq", bufs=3) as pgs, \
             tc.tile_pool(name="stg", bufs=4) as pstg, \
             tc.tile_pool(name="stats", bufs=2) as pst, \
             tc.tile_pool(name="gn", bufs=4) as pgn, \
             tc.tile_pool(name="gt", bufs=4) as pgt, \
             tc.tile_pool(name="hnscr", bufs=4) as phs, \
             tc.tile_pool(name="psA", bufs=3, space="PSUM") as ppsa, \
             tc.tile_pool(name="psU", bufs=2, space="PSUM") as ppsu, \
             tc.tile_pool(name="psIO", bufs=3, space="PSUM") as ppsio:

            def phase_a(j):
                jw = slice(j * nt, (j + 1) * nt)
                if j < 2:
                    xks = [px.tile([128, 2, nt], f8, tag=f"x0k{k}", bufs=2,
                                   name=f"xs{j}k{k}") for k in range(NPAIR)]
                    for k in range(NPAIR):
                        nc.sync.dma_start(xks[k][:],
                                          xtt.ap()[j, :, 2 * k:2 * k + 2, :])
                    xsl = [xks[k][:] for k in range(NPAIR)]
                else:
                    xs = px.tile([128, KD, nt], f8, tag="x", name=f"xs{j}")
                    nc.sync.dma_start(xs[:], xtt.ap()[j])
                    xsl = [xs[:, 2 * k:2 * k + 2, :] for k in range(NPAIR)]
                # mm1 (fp8 DoubleRow) + h1 relu store
                h1 = ph1.tile([128, KH, nt], bf16, tag="h1", name=f"h1_{j}")
                pms = []
                for m in range(KH):
                    pm = ppsa.tile([128, 1, nt], f32, tag="A", name=f"pa1_{j}_{m}")
                    for k in range(NPAIR):
                        nc.tensor.matmul(
                            pm[:, 0, :],
                            w1_sb[:, 2 * k:2 * k + 2, m * 128:(m + 1) * 128],
                            xsl[k], start=(k == 0), stop=(k == NPAIR - 1),
                            perf_mode=DR)
                    pms.append(pm)
                for m in range(KH):
                    nc.vector.tensor_scalar(
                        out=h1[:, m, :], in0=pms[m][:, 0, :],
                        scalar1=b1_sb[:, m:m + 1], scalar2=0.0,
                        op0=ALU.add, op1=ALU.max)
                # mm2 (bf16) + g store
                pm2s = []
                for m in range(KH):
                    pm = ppsa.tile([128, 1, nt], f32, tag="A", name=f"pa2_{j}_{m}")
                    for k in range(KH):
                        nc.tensor.matmul(
                            pm[:, 0, :], w2_sb[:, k, m * 128:(m + 1) * 128],
                            h1[:, k, :], start=(k == 0), stop=(k == KH - 1))
                    pm2s.append(pm)
                for m in range(KH):
                    nc.vector.tensor_scalar(
                        out=gbuf[:, m, jw], in0=pm2s[m][:, 0, :],
                        scalar1=1.0 / 256.0, scalar2=b2_sb[:, m:m + 1],
                        op0=ALU.mult, op1=ALU.add)
                # gsq + stats matmuls (bf16) -> {mu, msq} -> sbuf -> DRAM
                gsq = pgs.tile([128, KH, nt], bf16, tag="gsq", name=f"gsq{j}")
                nc.vector.tensor_tensor(out=gsq[:], in0=gbuf[:, :, jw],
                                        in1=gbuf[:, :, jw], op=ALU.mult)
                pstat = ppsa.tile([128, 1, nt], f32, tag="A", name=f"pstat_{j}")
                for m in range(KH):
                    nc.tensor.matmul(pstat[0:32, 0, :], ones_sb[:],
                                     gbuf[:, m, jw],
                                     start=(m == 0), stop=(m == KH - 1))
                for m in range(KH):
                    nc.tensor.matmul(pstat[32:64, 0, :], ones_sb[:],
                                     gsq[:, m, :], skip_group_check=True,
                                     start=(m == 0), stop=(m == KH - 1))
                stg = pstg.tile([64, nt], bf16, tag="stg", name=f"stg{j}")
                nc.vector.tensor_copy(stg[:], pstat[0:64, 0, :])
                nc.gpsimd.dma_start(statsd[j], stg[31:33, :])

            def phase_stats(g, tl):
                ng = len(tl)
                j0 = tl[0]
                mu2 = pst.tile([ng, nt], bf16, tag="mu2", name=f"mu2g{g}")
                nc.gpsimd.dma_start(
                    mu2[:], statsd[j0:j0 + ng, 0:1, :]
                    .rearrange("j o t -> (j o) t"))
                ms2 = pst.tile([ng, nt], bf16, tag="ms2", name=f"ms2g{g}")
                nc.gpsimd.dma_start(
                    ms2[:], statsd[j0:j0 + ng, 1:2, :]
                    .rearrange("j o t -> (j o) t"))
                musq = pst.tile([ng, nt], f32, tag="musq", name=f"musqg{g}")
                nc.vector.tensor_tensor(out=musq[:], in0=mu2[:], in1=mu2[:],
                                        op=ALU.mult)
                varr = pst.tile([ng, nt], f32, tag="varr", name=f"varrg{g}")
                nc.vector.tensor_tensor(out=varr[:], in0=ms2[:], in1=musq[:],
                                        op=ALU.subtract)
                lnv = pst.tile([ng, nt], f32, tag="lnv", name=f"lnvg{g}")
                nc.scalar.activation(lnv[:], varr[:], AF.Ln,
                                     bias=eps_sb[0:ng, :])
                sst = pst.tile([ng, 2, nt], bf16, tag="sst", name=f"sstg{g}")
                nc.scalar.activation(sst[:, 0, :], lnv[:], AF.Exp, scale=-0.5,
                                     bias=ln8_sb[0:ng, :])
                nc.vector.tensor_tensor(out=sst[:, 1, :], in0=sst[:, 0, :],
                                        in1=mu2[:], op=ALU.mult)
                nc.gpsimd.dma_start(ssd[j0:j0 + ng], sst[:])

            def phase_b(j):
                jw = slice(j * nt, (j + 1) * nt)
                sb = pgn.tile([128, 2, nt], bf16, tag="sb", name=f"sb{j}")
                smb = pgn.tile([128, 2, nt], bf16, tag="smb", name=f"smb{j}")
                for m in range(KH):
                    nc.gpsimd.dma_start(
                        sb[:, m:m + 1, :],
                        ssd[j:j + 1, 0:1, :].partition_broadcast(128))
                    nc.gpsimd.dma_start(
                        smb[:, m:m + 1, :],
                        ssd[j:j + 1, 1:2, :].partition_broadcast(128))
                gs = pgn.tile([128, KH, nt], bf16, tag="gs", name=f"gs{j}")
                nc.vector.tensor_tensor(out=gs[:], in0=gbuf[:, :, jw],
                                        in1=sb[:], op=ALU.mult)
                gns = pgn.tile([128, KH, nt], f8, tag="gns", name=f"gns{j}")
                nc.vector.tensor_tensor(out=gns[:], in0=gs[:], in1=smb[:],
                                        op=ALU.subtract)
                # mm3 (fp8 DoubleRow): chunk order [i0,o0 | i1,o1 | u0,u1]
                tu = pgt.tile([128, KH, nt], bf16, tag="tu", name=f"tu{j}")
                sis = []
                for m in range(KH):
                    pu = ppsu.tile([128, 1, nt], f32, tag="U", name=f"pu{j}_{m}")
                    nc.tensor.matmul(
                        pu[:, 0, :],
                        w3_sb[:, :, (4 + m) * 128:(5 + m) * 128],
                        gns[:], start=True, stop=True, perf_mode=DR)
                    nc.scalar.activation(tu[:, m, :], pu[:, 0, :], AF.Tanh,
                                         bias=c3_sb[:, 4 + m:5 + m],
                                         scale=1.0 / 128.0)
                    pii = ppsio.tile([128, 1, nt], f32, tag="IO",
                                     name=f"pii{j}_{m}")
                    nc.tensor.matmul(
                        pii[:, 0, :],
                        w3_sb[:, :, (2 * m) * 128:(2 * m + 1) * 128],
                        gns[:], start=True, stop=True, perf_mode=DR)
                    pio = ppsio.tile([128, 1, nt], f32, tag="IO",
                                     name=f"pio{j}_{m}")
                    nc.tensor.matmul(
                        pio[:, 0, :],
                        w3_sb[:, :, (2 * m + 1) * 128:(2 * m + 2) * 128],
                        gns[:], start=True, stop=True, perf_mode=DR)
                    siso = pgt.tile([128, 2, nt], bf16, tag="siso",
                                    name=f"siso{j}_{m}")
                    nc.scalar.activation(siso[:, 0, :], pii[:, 0, :],
                                         AF.Sigmoid,
                                         bias=c3_sb[:, 2 * m:2 * m + 1],
                                         scale=1.0 / 128.0)
                    nc.scalar.activation(siso[:, 1, :], pio[:, 0, :],
                                         AF.Sigmoid,
                                         bias=c3_sb[:, 2 * m + 1:2 * m + 2],
                                         scale=1.0 / 128.0)
                    sis.append(siso)
                cp = pgt.tile([128, KH, nt], bf16, tag="cp", name=f"cp{j}")
                for m in range(KH):
                    nc.gpsimd.tensor_tensor(out=cp[:, m, :],
                                            in0=sis[m][:, 0, :],
                                            in1=tu[:, m, :], op=ALU.mult)
                tc_t = pgt.tile([128, KH, nt], bf16, tag="tc", name=f"tc{j}")
                nc.scalar.activation(tc_t[:], cp[:], AF.Tanh)
                for m in range(KH):
                    hs = phs.tile([128, nt], bf16, tag="hs", name=f"hs{j}_{m}")
                    nc.vector.scalar_tensor_tensor(
                        out=hs[:], in0=sis[m][:, 1, :], scalar=1.0,
                        in1=tc_t[:, m, :], op0=ALU.mult, op1=ALU.mult,
                        accum_out=accslab[:, m, j:j + 1])

            if ngroups == 2:
                # coarse 2-group software pipeline (uneven split)
                c0 = min(ntiles - 1, (ntiles * 3) // 5)
                g2 = [list(range(c0)), list(range(c0, ntiles))]
                for j in g2[0]:
                    phase_a(j)
                phase_stats(0, g2[0])
                prev, cur = g2[0], g2[1]
                for i in range(len(cur)):
                    phase_a(cur[i])
                    if i < len(prev):
                        phase_b(prev[i])
                phase_stats(1, cur)
                for i in range(len(cur), len(prev)):
                    phase_b(prev[i])
                for j in g2[1]:
                    phase_b(j)
            else:
                # pipeline: B(j-LAG) with A(j); stats(g) after its last A tile
                for j in range(ntiles):
                    if j >= LAG:
                        phase_b(j - LAG)
                    phase_a(j)
                    if j % SG == SG - 1 or j == ntiles - 1:
                        phase_stats(j // SG, groups[j // SG])
                for j in range(ntiles - LAG, ntiles):
                    phase_b(j)

        # ---------------- partial pooled sum -> DRAM ----------------
        with tc.tile_pool(name="fin", bufs=1) as pf:
            pv = pf.tile([128, KH, 1], f32)
            nc.vector.tensor_reduce(out=pv[:], in_=accslab[:],
                                    axis=mybir.AxisListType.X, op=ALU.add)
            nc.sync.dma_start(out_d.ap(), pv[:, :, 0])

    nc.compile()
    return nc


def host_prep(inputs, npc, nt, ncores):
    """Shard + lay out inputs for the device. Returns in_maps (list per core)."""
    bf16 = ml_dtypes.bfloat16
    f8 = ml_dtypes.float8_e4m3
    ntiles = npc // nt
    KH = H // 128

    x = np.asarray(inputs["x"], np.float32)
    W1 = np.asarray(inputs["W1"], np.float32)
    b1 = np.asarray(inputs["b1"], np.float32)
    W2 = np.asarray(inputs["W2"], np.float32)
    b2 = np.asarray(inputs["b2"], np.float32)
    ln_w = np.asarray(inputs["ln_w"], np.float32)
    ln_b = np.asarray(inputs["ln_b"], np.float32)
    W_iou = np.asarray(inputs["W_iou"], np.float32)
    b_wiou = np.asarray(inputs["b_wiou"], np.float32)
    b_uiou = np.asarray(inputs["b_uiou"], np.float32)

    assert np.allclose(b1, 0.0) and np.allclose(b2, 0.0), (
        "kernel assumes b1 == b2 == 0 (true for this problem's setup_inputs)")
    Wio = W_iou * ln_w[None, :]
    c3 = (W_iou @ ln_b + b_wiou + b_uiou).astype(np.float32)   # [3H]
    # device iou chunk order [i0, o0, i1, o1, u0, u1] (chunks of 128)
    chunk_order = [0, 2, 1, 3, 4, 5]
    Wio_r = Wio.reshape(6, 128, H)[chunk_order]         # [6,128,H]
    c3_r = c3.reshape(6, 128)[chunk_order]              # [6,128]

    shared = {
        "w1d": np.ascontiguousarray(
            (W1.T * 16.0).reshape(KH * 3, 128, H).transpose(1, 0, 2)
        ).astype(f8),
        "b1d": np.ascontiguousarray((b1 * 256.0).reshape(KH, 128).T),
        "w2d": np.ascontiguousarray(
            W2.T.reshape(KH, 128, H).transpose(1, 0, 2)).astype(bf16),
        "b2d": np.ascontiguousarray(b2.reshape(KH, 128).T),
        "w3d": np.ascontiguousarray(
            (Wio_r.transpose(2, 0, 1) * 16.0)       # [H, 6, 128]
            .reshape(KH, 128, 6 * 128).transpose(1, 0, 2)
        ).astype(f8),
        "c3d": np.ascontiguousarray(c3_r.T),        # [128, 6]
    }
    in_maps = []
    for c in range(ncores):
        xs = x[c * npc:(c + 1) * npc]
        xtt = ((xs * 16.0).reshape(ntiles, nt, D // 128, 128)
               .transpose(0, 3, 2, 1)).astype(f8)
        in_maps.append({"xtt": np.ascontiguousarray(xtt), **shared})
    return in_maps


def host_finish(results, inputs, ncores):
    """Sum per-core pooled partials, apply the classifier on host (fp32)."""
    acc = np.zeros((128, H // 128), np.float64)
    for c in range(ncores):
        acc += np.asarray(results[c]["out"], np.float64)
    pooled = acc.T.reshape(1, H).astype(np.float32) / float(N_TOTAL)
    Wc1 = np.asarray(inputs["Wc1"], np.float32)
    bc1 = np.asarray(inputs["bc1"], np.float32)
    Wc2 = np.asarray(inputs["Wc2"], np.float32)
    bc2 = np.asarray(inputs["bc2"], np.float32)
    z = np.maximum(pooled @ Wc1.T + bc1, 0.0)
    return np.ascontiguousarray((z @ Wc2.T + bc2).astype(np.float32))


def kernel(**inputs):
    from concourse.bass_utils import run_bass_kernel_spmd

    npc = N_TOTAL // NCORES
    nt = 500
    key = (npc, nt, NCORES)
    if key not in _CACHE:
        _CACHE[key] = build_nc(npc, nt, NCORES)
    nc = _CACHE[key]
    in_maps = host_prep(inputs, npc, nt, NCORES)
    res = run_bass_kernel_spmd(nc, in_maps, core_ids=list(range(NCORES)))
    return host_finish(res.results, inputs, NCORES)


# revision 41
# speedup vs baseline: 1.1631x; 1.1631x over previous
"""Trainium2 Bass kernel for nn_BaselineRvNNModel (collapsed RvNN/TreeLSTM).

Math (reference collapses to a per-node MLP + mean pool + classifier;
edge_index is dead):
    h1 = relu(x @ W1.T + b1)                      [N, H]
    g  = h1 @ W2.T + b2                           [N, H]   (pre-LN)
    gn = (g - mu) * rsqrt(var + eps)              per-row LN core
    iou = gn @ (W_iou*ln_w).T + (W_iou@ln_b + b_wiou + b_uiou)
    i, o, u = split(iou); c = sig(i)*tanh(u); hn = sig(o)*tanh(c)
    pooled = mean_rows(hn);  out = relu(pooled @ Wc1.T + bc1) @ Wc2.T + bc2

Distribution: data-parallel over nodes, 12500 rows/core on 8 cores. Each
core emits its partial pooled sum [128, 2] f32; the host sums the 8
partials and applies the tiny classifier (256->128->4) in fp32.

Precision: mm1 (x@W1.T) and mm3 (gn@Wio.T) run as fp8-e4m3 DoubleRow
matmuls; mm2 and the LN-stats matmuls stay bf16 (mm2 in fp8 dominates
the error budget). Measured rel err ~5e-3 (tolerance 2e-2).

Scales (powers of 2): x*16, W1*16 -> pm1 = 256*xW1; h1 stored as
256*relu bf16; g = pm2/256 + b2 true-scale bf16; s8 = 8*rsqrt(var+eps);
gns = fp8(g*s8 - 8*s*mu) = 8*gn; W3*16 -> pm3 = 128*iou; gates use ACT
scale=1/128 with per-chunk bias c3.
"""

import numpy as np
import ml_dtypes

N_TOTAL = 100000
D = 768
H = 256
C = 4
NCORES = 8
LN_EPS = 1e-5

_CACHE = {}


def build_nc(npc, nt, ncores, ngroups=2):
    from contextlib import ExitStack
    import concourse.bass as bass
    import concourse.bacc as bacc
    import concourse.tile as tile
    from concourse import mybir

    f32 = mybir.dt.float32
    bf16 = mybir.dt.bfloat16
    f8 = mybir.dt.float8e4
    AF = mybir.ActivationFunctionType
    ALU = mybir.AluOpType
    DR = mybir.MatmulPerfMode.DoubleRow

    ntiles = npc // nt
    assert ntiles * nt == npc
    KD = D // 128            # 6 contraction chunks for x
    KH = H // 128            # 2 chunks for H
    NPAIR = KD // 2          # 3 DoubleRow pairs for mm1

    nc = bacc.Bacc("TRN2", target_bir_lowering=False, debug=False,
                   num_devices=ncores)

    xtt = nc.dram_tensor("xtt", [ntiles, 128, KD, nt], f8, kind="ExternalInput")
    w1d = nc.dram_tensor("w1d", [128, KD, H], f8, kind="ExternalInput")
    b1d = nc.dram_tensor("b1d", [128, KH], f32, kind="ExternalInput")   # b1*256
    w2d = nc.dram_tensor("w2d", [128, KH, H], bf16, kind="ExternalInput")
    b2d = nc.dram_tensor("b2d", [128, KH], f32, kind="ExternalInput")
    w3d = nc.dram_tensor("w3d", [128, KH, 3 * H], f8, kind="ExternalInput")
    c3d = nc.dram_tensor("c3d", [128, 3 * KH], f32, kind="ExternalInput")
    out_d = nc.dram_tensor("out", [128, KH], f32, kind="ExternalOutput")

    with tile.TileContext(nc) as tc, ExitStack() as ctx:
        # ---------------- constants (live whole kernel) ----------------
        pconst = ctx.enter_context(tc.tile_pool(name="consts", bufs=1))
        w1_sb = pconst.tile([128, KD, H], f8)
        nc.sync.dma_start(w1_sb[:], w1d.ap())
        b1_sb = pconst.tile([128, KH], f32)
        nc.sync.dma_start(b1_sb[:], b1d.ap())
        w2_sb = pconst.tile([128, KH, H], bf16)
        nc.sync.dma_start(w2_sb[:], w2d.ap())
        b2_sb = pconst.tile([128, KH], f32)
        nc.sync.dma_start(b2_sb[:], b2d.ap())
        w3_sb = pconst.tile([128, KH, 3 * H], f8)
        nc.gpsimd.dma_start(w3_sb[:], w3d.ap())
        c3_sb = pconst.tile([128, 3 * KH], f32)
        nc.gpsimd.dma_start(c3_sb[:], c3d.ap())
        ones_sb = pconst.tile([128, 32], bf16)
        nc.vector.memset(ones_sb[:], 1.0 / H)
        eps_sb = pconst.tile([16, 1], f32)
        nc.vector.memset(eps_sb[:], LN_EPS)
        ln8_sb = pconst.tile([16, 1], f32)
        nc.vector.memset(ln8_sb[:], float(np.log(8.0)))

        # persistent buffers
        pg = ctx.enter_context(tc.tile_pool(name="gbuf", bufs=1))
        gbuf = pg.tile([128, KH, npc], bf16)
        accslab = pg.tile([128, KH, 32], f32)
        nc.vector.memset(accslab[:], 0.0)

        pdram = ctx.enter_context(tc.tile_pool(name="dram", bufs=1, space="DRAM"))
        statsd = pdram.tile([ntiles, 2, nt], bf16)   # per-tile {mu, msq}
        ssd = pdram.tile([ntiles, 2, nt], bf16)      # per-tile {8s, 8*s*mu}

        # fine-grained software pipeline: stats every SG tiles, B lags A by SG
        SG = ngroups if ngroups > 2 else 5
        LAG = SG
        groups = [list(range(g, min(g + SG, ntiles)))
                  for g in range(0, ntiles, SG)]

        with tc.tile_pool(name="xin", bufs=4) as px, \
             tc.tile_pool(name="h1", bufs=4) as ph1, \
             tc.tile_pool(name="gsq", bufs=4) as pgs, \
             tc.tile_pool(name="stg", bufs=6) as pstg, \
             tc.tile_pool(name="stats", bufs=2) as pst, \
             tc.tile_pool(name="gn", bufs=6) as pgn, \
             tc.tile_pool(name="gt", bufs=6) as pgt, \
             tc.tile_pool(name="hnscr", bufs=6) as phs, \
             tc.tile_pool(name="psA", bufs=3, space="PSUM") as ppsa, \
             tc.tile_pool(name="psU", bufs=1, space="PSUM") as ppsu, \
             tc.tile_pool(name="psIO", bufs=4, space="PSUM") as ppsio:

            def phase_a(j):
                jw = slice(j * nt, (j + 1) * nt)
                if j < 2:
                    xks = [px.tile([128, 2, nt], f8, tag=f"x0k{k}", bufs=2,
                                   name=f"xs{j}k{k}") for k in range(NPAIR)]
                    for k in range(NPAIR):
                        nc.sync.dma_start(xks[k][:],
                                          xtt.ap()[j, :, 2 * k:2 * k + 2, :])
                    xsl = [xks[k][:] for k in range(NPAIR)]
                else:
                    xs = px.tile([128, KD, nt], f8, tag="x", name=f"xs{j}")
                    nc.sync.dma_start(xs[:], xtt.ap()[j])
                    xsl = [xs[:, 2 * k:2 * k + 2, :] for k in range(NPAIR)]
                # mm1 (fp8 DoubleRow) + h1 relu store
                h1 = ph1.tile([128, KH, nt], bf16, tag="h1", name=f"h1_{j}")
                pms = []
                for m in range(KH):
                    pm = ppsa.tile([128, 1, nt], f32, tag="A", name=f"pa1_{j}_{m}")
                    for k in range(NPAIR):
                        nc.tensor.matmul(
                            pm[:, 0, :],
                            w1_sb[:, 2 * k:2 * k + 2, m * 128:(m + 1) * 128],
                            xsl[k], start=(k == 0), stop=(k == NPAIR - 1),
                            perf_mode=DR)
                    pms.append(pm)
                for m in range(KH):
                    nc.vector.tensor_scalar(
                        out=h1[:, m, :], in0=pms[m][:, 0, :],
                        scalar1=b1_sb[:, m:m + 1], scalar2=0.0,
                        op0=ALU.add, op1=ALU.max)
                # mm2 (bf16) + g store
                pm2s = []
                for m in range(KH):
                    pm = ppsa.tile([128, 1, nt], f32, tag="A", name=f"pa2_{j}_{m}")
                    for k in range(KH):
                        nc.tensor.matmul(
                            pm[:, 0, :], w2_sb[:, k, m * 128:(m + 1) * 128],
                            h1[:, k, :], start=(k == 0), stop=(k == KH - 1))
                    pm2s.append(pm)
                for m in range(KH):
                    nc.vector.tensor_scalar(
                        out=gbuf[:, m, jw], in0=pm2s[m][:, 0, :],
                        scalar1=1.0 / 256.0, scalar2=b2_sb[:, m:m + 1],
                        op0=ALU.mult, op1=ALU.add)
                # gsq + stats matmuls (bf16) -> {mu, msq} -> sbuf -> DRAM
                gsq = pgs.tile([128, KH, nt], bf16, tag="gsq", name=f"gsq{j}")
                nc.vector.tensor_tensor(out=gsq[:], in0=gbuf[:, :, jw],
                                        in1=gbuf[:, :, jw], op=ALU.mult)
                pstat = ppsa.tile([128, 1, nt], f32, tag="A", name=f"pstat_{j}")
                for m in range(KH):
                    nc.tensor.matmul(pstat[0:32, 0, :], ones_sb[:],
                                     gbuf[:, m, jw],
                                     start=(m == 0), stop=(m == KH - 1))
                for m in range(KH):
                    nc.tensor.matmul(pstat[32:64, 0, :], ones_sb[:],
                                     gsq[:, m, :], skip_group_check=True,
                                     start=(m == 0), stop=(m == KH - 1))
                stg = pstg.tile([64, nt], bf16, tag="stg", name=f"stg{j}")
                nc.vector.tensor_copy(stg[:], pstat[0:64, 0, :])
                nc.gpsimd.dma_start(statsd[j], stg[31:33, :])

            def phase_stats(g, tl):
                ng = len(tl)
                j0 = tl[0]
                mu2 = pst.tile([ng, nt], bf16, tag="mu2", name=f"mu2g{g}")
                nc.gpsimd.dma_start(
                    mu2[:], statsd[j0:j0 + ng, 0:1, :]
                    .rearrange("j o t -> (j o) t"))
                ms2 = pst.tile([ng, nt], bf16, tag="ms2", name=f"ms2g{g}")
                nc.gpsimd.dma_start(
                    ms2[:], statsd[j0:j0 + ng, 1:2, :]
                    .rearrange("j o t -> (j o) t"))
                musq = pst.tile([ng, nt], f32, tag="musq", name=f"musqg{g}")
                nc.vector.tensor_tensor(out=musq[:], in0=mu2[:], in1=mu2[:],
                                        op=ALU.mult)
                varr = pst.tile([ng, nt], f32, tag="varr", name=f"varrg{g}")
                nc.vector.tensor_tensor(out=varr[:], in0=ms2[:], in1=musq[:],
                                        op=ALU.subtract)
                lnv = pst.tile([ng, nt], f32, tag="lnv", name=f"lnvg{g}")
                nc.scalar.activation(lnv[:], varr[:], AF.Ln,
                                     bias=eps_sb[0:ng, :])
                sst = pst.tile([ng, 2, nt], bf16, tag="sst", name=f"sstg{g}")
                nc.scalar.activation(sst[:, 0, :], lnv[:], AF.Exp, scale=-0.5,
                                     bias=ln8_sb[0:ng, :])
                nc.vector.tensor_tensor(out=sst[:, 1, :], in0=sst[:, 0, :],
                                        in1=mu2[:], op=ALU.mult)
                nc.gpsimd.dma_start(ssd[j0:j0 + ng], sst[:])

            def phase_b(j):
                jw = slice(j * nt, (j + 1) * nt)
                sb = pgn.tile([128, 2, nt], bf16, tag="sb", name=f"sb{j}")
                smb = pgn.tile([128, 2, nt], bf16, tag="smb", name=f"smb{j}")
                for m in range(KH):
                    nc.gpsimd.dma_start(
                        sb[:, m:m + 1, :],
                        ssd[j:j + 1, 0:1, :].partition_broadcast(128))
                    nc.gpsimd.dma_start(
                        smb[:, m:m + 1, :],
                        ssd[j:j + 1, 1:2, :].partition_broadcast(128))
                gs = pgn.tile([128, KH, nt], bf16, tag="gs", name=f"gs{j}")
                nc.vector.tensor_tensor(out=gs[:], in0=gbuf[:, :, jw],
                                        in1=sb[:], op=ALU.mult)
                gns = pgn.tile([128, KH, nt], f8, tag="gns", name=f"gns{j}")
                nc.vector.tensor_tensor(out=gns[:], in0=gs[:], in1=smb[:],
                                        op=ALU.subtract)
                # mm3 (fp8 DoubleRow): chunk order [i0,o0 | i1,o1 | u0,u1]
                tu = pgt.tile([128, KH, nt], bf16, tag="tu", name=f"tu{j}")
                sis = []
                for m in range(KH):
                    pu = ppsu.tile([128, 1, nt], f32, tag="U", name=f"pu{j}_{m}")
                    nc.tensor.matmul(
                        pu[:, 0, :],
                        w3_sb[:, :, (4 + m) * 128:(5 + m) * 128],
                        gns[:], start=True, stop=True, perf_mode=DR)
                    nc.scalar.activation(tu[:, m, :], pu[:, 0, :], AF.Tanh,
                                         bias=c3_sb[:, 4 + m:5 + m],
                                         scale=1.0 / 128.0)
                    pii = ppsio.tile([128, 1, nt], f32, tag="IO",
                                     name=f"pii{j}_{m}")
                    nc.tensor.matmul(
                        pii[:, 0, :],
                        w3_sb[:, :, (2 * m) * 128:(2 * m + 1) * 128],
                        gns[:], start=True, stop=True, perf_mode=DR)
                    pio = ppsio.tile([128, 1, nt], f32, tag="IO",
                                     name=f"pio{j}_{m}")
                    nc.tensor.matmul(
                        pio[:, 0, :],
                        w3_sb[:, :, (2 * m + 1) * 128:(2 * m + 2) * 128],
                        gns[:], start=True, stop=True, perf_mode=DR)
                    siso = pgt.tile([128, 2, nt], bf16, tag="siso",
                                    name=f"siso{j}_{m}")
                    nc.scalar.activation(siso[:, 0, :], pii[:, 0, :],
                                         AF.Sigmoid,
                                         bias=c3_sb[:, 2 * m:2 * m + 1],
                                         scale=1.0 / 128.0)
                    nc.scalar.activation(siso[:, 1, :], pio[:, 0, :],
                                         AF.Sigmoid,
                                         bias=c3_sb[:, 2 * m + 1:2 * m + 2],
                                         scale=1.0 / 128.0)
                    sis.append(siso)
                cp = pgt.tile([128, KH, nt], bf16, tag="cp", name=f"cp{j}")
                for m in range(KH):
                    nc.gpsimd.tensor_tensor(out=cp[:, m, :],
                                            in0=sis[m][:, 0, :],
                                            in1=tu[:, m, :], op=ALU.mult)
                tc_t = pgt.tile([128, KH, nt], bf16, tag="tc", name=f"tc{j}")
                nc.scalar.activation(tc_t[:], cp[:], AF.Tanh)
                for m in range(KH):
                    hs = phs.tile([128, nt], bf16, tag="hs", name=f"hs{j}_{m}")
                    nc.vector.scalar_tensor_tensor(
                        out=hs[:], in0=sis[m][:, 1, :], scalar=1.0,
                        in1=tc_t[:, m, :], op0=ALU.mult, op1=ALU.mult,
                        accum_out=accslab[:, m, j:j + 1])

            if ngroups == 2:
                # coarse 2-group software pipeline (uneven split)
                c0 = min(ntiles - 1, (ntiles * 3) // 5)
                g2 = [list(range(c0)), list(range(c0, ntiles))]
                for j in g2[0]:
                    phase_a(j)
                phase_stats(0, g2[0])
                prev, cur = g2[0], g2[1]
                for i in range(len(cur)):
                    phase_a(cur[i])
                    if i < len(prev):
                        phase_b(prev[i])
                phase_stats(1, cur)
                for i in range(len(cur), len(prev)):
                    phase_b(prev[i])
                for j in g2[1]:
                    phase_b(j)
            else:
                # pipeline: B(j-LAG) with A(j); stats(g) after its last A tile
                for j in range(ntiles):
                    if j >= LAG:
                        phase_b(j - LAG)
                    phase_a(j)
                    if j % SG == SG - 1 or j == ntiles - 1:
                        phase_stats(j // SG, groups[j // SG])
                for j in range(ntiles - LAG, ntiles):
                    phase_b(j)

        # ---------------- partial pooled sum -> DRAM ----------------
        with tc.tile_pool(name="fin", bufs=1) as pf:
            pv = pf.tile([128, KH, 1], f32)
            nc.vector.tensor_reduce(out=pv[:], in_=accslab[:],
                                    axis=mybir.AxisListType.X, op=ALU.add)
            nc.sync.dma_start(out_d.ap(), pv[:, :, 0])

    nc.compile()
    return nc


def host_prep(inputs, npc, nt, ncores):
    """Shard + lay out inputs for the device. Returns in_maps (list per core)."""
    bf16 = ml_dtypes.bfloat16
    f8 = ml_dtypes.float8_e4m3
    ntiles = npc // nt
    KH = H // 128

    x = np.asarray(inputs["x"], np.float32)
    W1 = np.asarray(inputs["W1"], np.float32)
    b1 = np.asarray(inputs["b1"], np.float32)
    W2 = np.asarray(inputs["W2"], np.float32)
    b2 = np.asarray(inputs["b2"], np.float32)
    ln_w = np.asarray(inputs["ln_w"], np.float32)
    ln_b = np.asarray(inputs["ln_b"], np.float32)
    W_iou = np.asarray(inputs["W_iou"], np.float32)
    b_wiou = np.asarray(inputs["b_wiou"], np.float32)
    b_uiou = np.asarray(inputs["b_uiou"], np.float32)

    assert np.allclose(b1, 0.0) and np.allclose(b2, 0.0), (
        "kernel assumes b1 == b2 == 0 (true for this problem's setup_inputs)")
    Wio = W_iou * ln_w[None, :]
    c3 = (W_iou @ ln_b + b_wiou + b_uiou).astype(np.float32)   # [3H]
    # device iou chunk order [i0, o0, i1, o1, u0, u1] (chunks of 128)
    chunk_order = [0, 2, 1, 3, 4, 5]
    Wio_r = Wio.reshape(6, 128, H)[chunk_order]         # [6,128,H]
    c3_r = c3.reshape(6, 128)[chunk_order]              # [6,128]

    shared = {
        "w1d": np.ascontiguousarray(
            (W1.T * 16.0).reshape(KH * 3, 128, H).transpose(1, 0, 2)
        ).astype(f8),
        "b1d": np.ascontiguousarray((b1 * 256.0).reshape(KH, 128).T),
        "w2d": np.ascontiguousarray(
            W2.T.reshape(KH, 128, H).transpose(1, 0, 2)).astype(bf16),
        "b2d": np.ascontiguousarray(b2.reshape(KH, 128).T),
        "w3d": np.ascontiguousarray(
            (Wio_r.transpose(2, 0, 1) * 16.0)       # [H, 6, 128]
            .reshape(KH, 128, 6 * 128).transpose(1, 0, 2)
        ).astype(f8),
        "c3d": np.ascontiguousarray(c3_r.T),        # [128, 6]
    }
    in_maps = []
    for c in range(ncores):
        xs = x[c * npc:(c + 1) * npc]
        xtt = ((xs * 16.0).reshape(ntiles, nt, D // 128, 128)
               .transpose(0, 3, 2, 1)).astype(f8)
        in_maps.append({"xtt": np.ascontiguousarray(xtt), **shared})
    return in_maps


def host_finish(results, inputs, ncores):
    """Sum per-core pooled partials, apply the classifier on host (fp32)."""
    acc = np.zeros((128, H // 128), np.float64)
    for c in range(ncores):
        acc += np.asarray(results[c]["out"], np.float64)
    pooled = acc.T.reshape(1, H).astype(np.float32) / float(N_TOTAL)
    Wc1 = np.asarray(inputs["Wc1"], np.float32)
    bc1 = np.asarray(inputs["bc1"], np.float32)
    Wc2 = np.asarray(inputs["Wc2"], np.float32)
    bc2 = np.asarray(inputs["bc2"], np.float32)
    z = np.maximum(pooled @ Wc1.T + bc1, 0.0)
    return np.ascontiguousarray((z @ Wc2.T + bc2).astype(np.float32))


def kernel(**inputs):
    from concourse.bass_utils import run_bass_kernel_spmd

    npc = N_TOTAL // NCORES
    nt = 500
    key = (npc, nt, NCORES)
    if key not in _CACHE:
        _CACHE[key] = build_nc(npc, nt, NCORES)
    nc = _CACHE[key]
    in_maps = host_prep(inputs, npc, nt, NCORES)
    res = run_bass_kernel_spmd(nc, in_maps, core_ids=list(range(NCORES)))
    return host_finish(res.results, inputs, NCORES)


# revision 44
# speedup vs baseline: 1.1836x; 1.0177x over previous
"""Trainium2 Bass kernel for nn_BaselineRvNNModel (collapsed RvNN/TreeLSTM).

Math (reference collapses to a per-node MLP + mean pool + classifier;
edge_index is dead):
    h1 = relu(x @ W1.T + b1)                      [N, H]
    g  = h1 @ W2.T + b2                           [N, H]   (pre-LN)
    gn = (g - mu) * rsqrt(var + eps)              per-row LN core
    iou = gn @ (W_iou*ln_w).T + (W_iou@ln_b + b_wiou + b_uiou)
    i, o, u = split(iou); c = sig(i)*tanh(u); hn = sig(o)*tanh(c)
    pooled = mean_rows(hn);  out = relu(pooled @ Wc1.T + bc1) @ Wc2.T + bc2

Distribution: data-parallel over nodes, 12500 rows/core on 8 cores. Each
core emits its partial pooled sum [128, 2] f32; the host sums the 8
partials and applies the tiny classifier (256->128->4) in fp32.

Precision: mm1 (x@W1.T) and mm3 (gn@Wio.T) run as fp8-e4m3 DoubleRow
matmuls; mm2 and the LN-stats matmuls stay bf16 (mm2 in fp8 dominates
the error budget). Measured rel err ~5e-3 (tolerance 2e-2).

Scales (powers of 2): x*16, W1*16 -> pm1 = 256*xW1; h1 stored as
256*relu bf16; g = pm2/256 + b2 true-scale bf16; s8 = 8*rsqrt(var+eps);
gns = fp8(g*s8 - 8*s*mu) = 8*gn; W3*16 -> pm3 = 128*iou; gates use ACT
scale=1/128 with per-chunk bias c3.
"""

import numpy as np
import ml_dtypes

N_TOTAL = 100000
D = 768
H = 256
C = 4
NCORES = 8
LN_EPS = 1e-5

_CACHE = {}


def build_nc(npc, nt, ncores, ngroups=2):
    from contextlib import ExitStack
    import concourse.bass as bass
    import concourse.bacc as bacc
    import concourse.tile as tile
    from concourse import mybir

    f32 = mybir.dt.float32
    bf16 = mybir.dt.bfloat16
    f8 = mybir.dt.float8e4
    AF = mybir.ActivationFunctionType
    ALU = mybir.AluOpType
    DR = mybir.MatmulPerfMode.DoubleRow

    ntiles = npc // nt
    assert ntiles * nt == npc
    KD = D // 128            # 6 contraction chunks for x
    KH = H // 128            # 2 chunks for H
    NPAIR = KD // 2          # 3 DoubleRow pairs for mm1

    nc = bacc.Bacc("TRN2", target_bir_lowering=False, debug=False,
                   num_devices=ncores)

    xtt = nc.dram_tensor("xtt", [ntiles, 128, KD, nt], f8, kind="ExternalInput")
    w1d = nc.dram_tensor("w1d", [128, KD, H], f8, kind="ExternalInput")
    b1d = nc.dram_tensor("b1d", [128, KH], f32, kind="ExternalInput")   # b1*256
    w2d = nc.dram_tensor("w2d", [128, KH, H], bf16, kind="ExternalInput")
    b2d = nc.dram_tensor("b2d", [128, KH], f32, kind="ExternalInput")
    w3d = nc.dram_tensor("w3d", [128, KH, 3 * H], f8, kind="ExternalInput")
    c3d = nc.dram_tensor("c3d", [128, 3 * KH], f32, kind="ExternalInput")
    out_d = nc.dram_tensor("out", [128, KH], f32, kind="ExternalOutput")

    with tile.TileContext(nc) as tc, ExitStack() as ctx:
        # ---------------- constants (live whole kernel) ----------------
        pconst = ctx.enter_context(tc.tile_pool(name="consts", bufs=1))
        w1_sb = pconst.tile([128, KD, H], f8)
        nc.sync.dma_start(w1_sb[:], w1d.ap())
        b1_sb = pconst.tile([128, KH], f32)
        nc.sync.dma_start(b1_sb[:], b1d.ap())
        w2_sb = pconst.tile([128, KH, H], bf16)
        nc.sync.dma_start(w2_sb[:], w2d.ap())
        b2_sb = pconst.tile([128, KH], f32)
        nc.sync.dma_start(b2_sb[:], b2d.ap())
        w3_sb = pconst.tile([128, KH, 3 * H], f8)
        nc.gpsimd.dma_start(w3_sb[:], w3d.ap())
        c3_sb = pconst.tile([128, 3 * KH], f32)
        nc.gpsimd.dma_start(c3_sb[:], c3d.ap())
        ones_sb = pconst.tile([128, 32], bf16)
        nc.vector.memset(ones_sb[:], 1.0 / H)
        eps_sb = pconst.tile([16, 1], f32)
        nc.vector.memset(eps_sb[:], LN_EPS)
        ln8_sb = pconst.tile([16, 1], f32)
        nc.vector.memset(ln8_sb[:], float(np.log(8.0)))

        # persistent buffers
        pg = ctx.enter_context(tc.tile_pool(name="gbuf", bufs=1))
        gbuf = pg.tile([128, KH, npc], bf16)
        accslab = pg.tile([128, KH, 32], f32)
        nc.vector.memset(accslab[:], 0.0)

        pdram = ctx.enter_context(tc.tile_pool(name="dram", bufs=1, space="DRAM"))
        statsd = pdram.tile([ntiles, 2, nt], bf16)   # per-tile {mu, msq}
        ssd = pdram.tile([ntiles, 2, nt], bf16)      # per-tile {8s, 8*s*mu}

        # fine-grained software pipeline: stats every SG tiles, B lags A by SG
        SG = ngroups if ngroups > 2 else 5
        LAG = SG
        groups = [list(range(g, min(g + SG, ntiles)))
                  for g in range(0, ntiles, SG)]

        with tc.tile_pool(name="xin", bufs=4) as px, \
             tc.tile_pool(name="h1", bufs=4) as ph1, \
             tc.tile_pool(name="gsq", bufs=4) as pgs, \
             tc.tile_pool(name="stg", bufs=6) as pstg, \
             tc.tile_pool(name="stats", bufs=2) as pst, \
             tc.tile_pool(name="gn", bufs=6) as pgn, \
             tc.tile_pool(name="gt", bufs=6) as pgt, \
             tc.tile_pool(name="hnscr", bufs=6) as phs, \
             tc.tile_pool(name="psA", bufs=3, space="PSUM") as ppsa, \
             tc.tile_pool(name="psU", bufs=1, space="PSUM") as ppsu, \
             tc.tile_pool(name="psIO", bufs=4, space="PSUM") as ppsio:

            def phase_a(j):
                jw = slice(j * nt, (j + 1) * nt)
                if j < 2:
                    xks = [px.tile([128, 2, nt], f8, tag=f"x0k{k}", bufs=2,
                                   name=f"xs{j}k{k}") for k in range(NPAIR)]
                    for k in range(NPAIR):
                        nc.sync.dma_start(xks[k][:],
                                          xtt.ap()[j, :, 2 * k:2 * k + 2, :])
                    xsl = [xks[k][:] for k in range(NPAIR)]
                else:
                    xs = px.tile([128, KD, nt], f8, tag="x", name=f"xs{j}")
                    nc.sync.dma_start(xs[:], xtt.ap()[j])
                    xsl = [xs[:, 2 * k:2 * k + 2, :] for k in range(NPAIR)]
                # mm1 (fp8 DoubleRow) + h1 relu store
                h1 = ph1.tile([128, KH, nt], bf16, tag="h1", name=f"h1_{j}")
                pms = []
                for m in range(KH):
                    pm = ppsa.tile([128, 1, nt], f32, tag="A", name=f"pa1_{j}_{m}")
                    for k in range(NPAIR):
                        nc.tensor.matmul(
                            pm[:, 0, :],
                            w1_sb[:, 2 * k:2 * k + 2, m * 128:(m + 1) * 128],
                            xsl[k], start=(k == 0), stop=(k == NPAIR - 1),
                            perf_mode=DR)
                    pms.append(pm)
                with tc.high_priority(offset=200):
                    for m in range(KH):
                        nc.vector.tensor_scalar(
                            out=h1[:, m, :], in0=pms[m][:, 0, :],
                            scalar1=b1_sb[:, m:m + 1], scalar2=0.0,
                            op0=ALU.add, op1=ALU.max)
                # mm2 (bf16) + g store
                pm2s = []
                for m in range(KH):
                    pm = ppsa.tile([128, 1, nt], f32, tag="A", name=f"pa2_{j}_{m}")
                    for k in range(KH):
                        nc.tensor.matmul(
                            pm[:, 0, :], w2_sb[:, k, m * 128:(m + 1) * 128],
                            h1[:, k, :], start=(k == 0), stop=(k == KH - 1))
                    pm2s.append(pm)
                with tc.high_priority(offset=200):
                    for m in range(KH):
                        nc.vector.tensor_scalar(
                            out=gbuf[:, m, jw], in0=pm2s[m][:, 0, :],
                            scalar1=1.0 / 256.0, scalar2=b2_sb[:, m:m + 1],
                            op0=ALU.mult, op1=ALU.add)
                # gsq + stats matmuls (bf16) -> {mu, msq} -> sbuf -> DRAM
                gsq = pgs.tile([128, KH, nt], bf16, tag="gsq", name=f"gsq{j}")
                nc.vector.tensor_tensor(out=gsq[:], in0=gbuf[:, :, jw],
                                        in1=gbuf[:, :, jw], op=ALU.mult)
                pstat = ppsa.tile([128, 1, nt], f32, tag="A", name=f"pstat_{j}")
                for m in range(KH):
                    nc.tensor.matmul(pstat[0:32, 0, :], ones_sb[:],
                                     gbuf[:, m, jw],
                                     start=(m == 0), stop=(m == KH - 1))
                for m in range(KH):
                    nc.tensor.matmul(pstat[32:64, 0, :], ones_sb[:],
                                     gsq[:, m, :], skip_group_check=True,
                                     start=(m == 0), stop=(m == KH - 1))
                stg = pstg.tile([64, nt], bf16, tag="stg", name=f"stg{j}")
                with tc.high_priority(offset=200):
                    nc.vector.tensor_copy(stg[:], pstat[0:64, 0, :])
                nc.gpsimd.dma_start(statsd[j], stg[31:33, :])

            def phase_stats(g, tl):
                ng = len(tl)
                j0 = tl[0]
                mu2 = pst.tile([ng, nt], bf16, tag="mu2", name=f"mu2g{g}")
                nc.gpsimd.dma_start(
                    mu2[:], statsd[j0:j0 + ng, 0:1, :]
                    .rearrange("j o t -> (j o) t"))
                ms2 = pst.tile([ng, nt], bf16, tag="ms2", name=f"ms2g{g}")
                nc.gpsimd.dma_start(
                    ms2[:], statsd[j0:j0 + ng, 1:2, :]
                    .rearrange("j o t -> (j o) t"))
                musq = pst.tile([ng, nt], f32, tag="musq", name=f"musqg{g}")
                nc.vector.tensor_tensor(out=musq[:], in0=mu2[:], in1=mu2[:],
                                        op=ALU.mult)
                varr = pst.tile([ng, nt], f32, tag="varr", name=f"varrg{g}")
                nc.vector.tensor_tensor(out=varr[:], in0=ms2[:], in1=musq[:],
                                        op=ALU.subtract)
                lnv = pst.tile([ng, nt], f32, tag="lnv", name=f"lnvg{g}")
                nc.scalar.activation(lnv[:], varr[:], AF.Ln,
                                     bias=eps_sb[0:ng, :])
                sst = pst.tile([ng, 2, nt], bf16, tag="sst", name=f"sstg{g}")
                nc.scalar.activation(sst[:, 0, :], lnv[:], AF.Exp, scale=-0.5,
                                     bias=ln8_sb[0:ng, :])
                nc.vector.tensor_tensor(out=sst[:, 1, :], in0=sst[:, 0, :],
                                        in1=mu2[:], op=ALU.mult)
                nc.gpsimd.dma_start(ssd[j0:j0 + ng], sst[:])

            def phase_b(j):
                jw = slice(j * nt, (j + 1) * nt)
                sb = pgn.tile([128, 2, nt], bf16, tag="sb", name=f"sb{j}")
                smb = pgn.tile([128, 2, nt], bf16, tag="smb", name=f"smb{j}")
                for m in range(KH):
                    nc.gpsimd.dma_start(
                        sb[:, m:m + 1, :],
                        ssd[j:j + 1, 0:1, :].partition_broadcast(128))
                    nc.gpsimd.dma_start(
                        smb[:, m:m + 1, :],
                        ssd[j:j + 1, 1:2, :].partition_broadcast(128))
                gs = pgn.tile([128, KH, nt], bf16, tag="gs", name=f"gs{j}")
                nc.vector.tensor_tensor(out=gs[:], in0=gbuf[:, :, jw],
                                        in1=sb[:], op=ALU.mult)
                gns = pgn.tile([128, KH, nt], f8, tag="gns", name=f"gns{j}")
                nc.vector.tensor_tensor(out=gns[:], in0=gs[:], in1=smb[:],
                                        op=ALU.subtract)
                # mm3 (fp8 DoubleRow): chunk order [i0,o0 | i1,o1 | u0,u1]
                tu = pgt.tile([128, KH, nt], bf16, tag="tu", name=f"tu{j}")
                sis = []
                for m in range(KH):
                    pu = ppsu.tile([128, 1, nt], f32, tag="U", name=f"pu{j}_{m}")
                    nc.tensor.matmul(
                        pu[:, 0, :],
                        w3_sb[:, :, (4 + m) * 128:(5 + m) * 128],
                        gns[:], start=True, stop=True, perf_mode=DR)
                    nc.scalar.activation(tu[:, m, :], pu[:, 0, :], AF.Tanh,
                                         bias=c3_sb[:, 4 + m:5 + m],
                                         scale=1.0 / 128.0)
                    pii = ppsio.tile([128, 1, nt], f32, tag="IO",
                                     name=f"pii{j}_{m}")
                    nc.tensor.matmul(
                        pii[:, 0, :],
                        w3_sb[:, :, (2 * m) * 128:(2 * m + 1) * 128],
                        gns[:], start=True, stop=True, perf_mode=DR)
                    pio = ppsio.tile([128, 1, nt], f32, tag="IO",
                                     name=f"pio{j}_{m}")
                    nc.tensor.matmul(
                        pio[:, 0, :],
                        w3_sb[:, :, (2 * m + 1) * 128:(2 * m + 2) * 128],
                        gns[:], start=True, stop=True, perf_mode=DR)
                    siso = pgt.tile([128, 2, nt], bf16, tag="siso",
                                    name=f"siso{j}_{m}")
                    nc.scalar.activation(siso[:, 0, :], pii[:, 0, :],
                                         AF.Sigmoid,
                                         bias=c3_sb[:, 2 * m:2 * m + 1],
                                         scale=1.0 / 128.0)
                    nc.scalar.activation(siso[:, 1, :], pio[:, 0, :],
                                         AF.Sigmoid,
                                         bias=c3_sb[:, 2 * m + 1:2 * m + 2],
                                         scale=1.0 / 128.0)
                    sis.append(siso)
                cp = pgt.tile([128, KH, nt], bf16, tag="cp", name=f"cp{j}")
                for m in range(KH):
                    nc.gpsimd.tensor_tensor(out=cp[:, m, :],
                                            in0=sis[m][:, 0, :],
                                            in1=tu[:, m, :], op=ALU.mult)
                tc_t = pgt.tile([128, KH, nt], bf16, tag="tc", name=f"tc{j}")
                nc.scalar.activation(tc_t[:], cp[:], AF.Tanh)
                for m in range(KH):
                    hs = phs.tile([128, nt], bf16, tag="hs", name=f"hs{j}_{m}")
                    nc.vector.scalar_tensor_tensor(
                        out=hs[:], in0=sis[m][:, 1, :], scalar=1.0,
                        in1=tc_t[:, m, :], op0=ALU.mult, op1=ALU.mult,
                        accum_out=accslab[:, m, j:j + 1])

            if ngroups == 2:
                # coarse 2-group software pipeline (uneven split)
                c0 = min(ntiles - 1, (ntiles * 3) // 5)
                g2 = [list(range(c0)), list(range(c0, ntiles))]
                for j in g2[0]:
                    phase_a(j)
                phase_stats(0, g2[0])
                prev, cur = g2[0], g2[1]
                for i in range(len(cur)):
                    phase_a(cur[i])
                    if i < len(prev):
                        phase_b(prev[i])
                phase_stats(1, cur)
                for i in range(len(cur), len(prev)):
                    phase_b(prev[i])
                for j in g2[1]:
                    phase_b(j)
            else:
                # pipeline: B(j-LAG) with A(j); stats(g) after its last A tile
                for j in range(ntiles):
                    if j >= LAG:
                        phase_b(j - LAG)
                    phase_a(j)
                    if j % SG == SG - 1 or j == ntiles - 1:
                        phase_stats(j // SG, groups[j // SG])
                for j in range(ntiles - LAG, ntiles):
                    phase_b(j)

        # ---------------- partial pooled sum -> DRAM ----------------
        with tc.tile_pool(name="fin", bufs=1) as pf:
            pv = pf.tile([128, KH, 1], f32)
            nc.vector.tensor_reduce(out=pv[:], in_=accslab[:],
                                    axis=mybir.AxisListType.X, op=ALU.add)
            nc.sync.dma_start(out_d.ap(), pv[:, :, 0])

    nc.compile()
    return nc


def host_prep(inputs, npc, nt, ncores):
    """Shard + lay out inputs for the device. Returns in_maps (list per core)."""
    bf16 = ml_dtypes.bfloat16
    f8 = ml_dtypes.float8_e4m3
    ntiles = npc // nt
    KH = H // 128

    x = np.asarray(inputs["x"], np.float32)
    W1 = np.asarray(inputs["W1"], np.float32)
    b1 = np.asarray(inputs["b1"], np.float32)
    W2 = np.asarray(inputs["W2"], np.float32)
    b2 = np.asarray(inputs["b2"], np.float32)
    ln_w = np.asarray(inputs["ln_w"], np.float32)
    ln_b = np.asarray(inputs["ln_b"], np.float32)
    W_iou = np.asarray(inputs["W_iou"], np.float32)
    b_wiou = np.asarray(inputs["b_wiou"], np.float32)
    b_uiou = np.asarray(inputs["b_uiou"], np.float32)

    assert np.allclose(b1, 0.0) and np.allclose(b2, 0.0), (
        "kernel assumes b1 == b2 == 0 (true for this problem's setup_inputs)")
    Wio = W_iou * ln_w[None, :]
    c3 = (W_iou @ ln_b + b_wiou + b_uiou).astype(np.float32)   # [3H]
    # device iou chunk order [i0, o0, i1, o1, u0, u1] (chunks of 128)
    chunk_order = [0, 2, 1, 3, 4, 5]
    Wio_r = Wio.reshape(6, 128, H)[chunk_order]         # [6,128,H]
    c3_r = c3.reshape(6, 128)[chunk_order]              # [6,128]

    shared = {
        "w1d": np.ascontiguousarray(
            (W1.T * 16.0).reshape(KH * 3, 128, H).transpose(1, 0, 2)
        ).astype(f8),
        "b1d": np.ascontiguousarray((b1 * 256.0).reshape(KH, 128).T),
        "w2d": np.ascontiguousarray(
            W2.T.reshape(KH, 128, H).transpose(1, 0, 2)).astype(bf16),
        "b2d": np.ascontiguousarray(b2.reshape(KH, 128).T),
        "w3d": np.ascontiguousarray(
            (Wio_r.transpose(2, 0, 1) * 16.0)       # [H, 6, 128]
            .reshape(KH, 128, 6 * 128).transpose(1, 0, 2)
        ).astype(f8),
        "c3d": np.ascontiguousarray(c3_r.T),        # [128, 6]
    }
    in_maps = []
    for c in range(ncores):
        xs = x[c * npc:(c + 1) * npc]
        xtt = ((xs * 16.0).reshape(ntiles, nt, D // 128, 128)
               .transpose(0, 3, 2, 1)).astype(f8)
        in_maps.append({"xtt": np.ascontiguousarray(xtt), **shared})
    return in_maps


def host_finish(results, inputs, ncores):
    """Sum per-core pooled partials, apply the classifier on host (fp32)."""
    acc = np.zeros((128, H // 128), np.float64)
    for c in range(ncores):
        acc += np.asarray(results[c]["out"], np.float64)
    pooled = acc.T.reshape(1, H).astype(np.float32) / float(N_TOTAL)
    Wc1 = np.asarray(inputs["Wc1"], np.float32)
    bc1 = np.asarray(inputs["bc1"], np.float32)
    Wc2 = np.asarray(inputs["Wc2"], np.float32)
    bc2 = np.asarray(inputs["bc2"], np.float32)
    z = np.maximum(pooled @ Wc1.T + bc1, 0.0)
    return np.ascontiguousarray((z @ Wc2.T + bc2).astype(np.float32))


def kernel(**inputs):
    from concourse.bass_utils import run_bass_kernel_spmd

    npc = N_TOTAL // NCORES
    nt = 500
    key = (npc, nt, NCORES)
    if key not in _CACHE:
        _CACHE[key] = build_nc(npc, nt, NCORES)
    nc = _CACHE[key]
    in_maps = host_prep(inputs, npc, nt, NCORES)
    res = run_bass_kernel_spmd(nc, in_maps, core_ids=list(range(NCORES)))
    return host_finish(res.results, inputs, NCORES)
